# revision 12
# baseline (speedup 1.0000x reference)
"""Criss-cross attention kernel for Trainium2, 8-core SPMD.

Sharding: batch (4) x head-group (2 of 4 heads each) -> 8 cores. Each core
computes the 1x1 conv projections for its 256 output channels (bf16 matmuls,
fp32 accum), the criss-cross attention for its 4 heads, and the residual
epilogue, returning out[b, ch0:ch0+256] exactly.

Per-core pipeline (per head-pair mt in {0,1}):
  conv:    t/f/g [128ch, 9216pos] bf16 via K=512 matmuls, bias via ACT evac
  gT:      PE transposes of g into [x,(y,d)] and [y,(x,d)] layouts
  row att: per y: e_rowT = f_y^T t_y -> exp (no max-sub; logits are ~ +-6)
           -> out_row = gT_row^T ER, Z-rows via ones-matmul columns
  col att: diag mask via PSUM-preload matmul, e_colT = f_x^T t_x, exp,
           out_col accumulated into outRC, Z-map = transpose(Zrow) + cols
  epilogue: Zinv=gamma/Z broadcast via K=1 matmul, out = outRC*Zb + v
"""

import numpy as np
import ml_dtypes

import concourse.bass as bass
import concourse.mybir as mybir
from concourse.tile import TileContext

BF = ml_dtypes.bfloat16
F32 = mybir.dt.float32
BF16 = mybir.dt.bfloat16
AF = mybir.ActivationFunctionType
ALU = mybir.AluOpType

B, C, H, W = 4, 512, 96, 96
HW = H * W
D = 64            # head dim
G = 4             # y/x blocks per psum group
NG = H // G       # 24 groups
CCH = 512         # conv position chunk
NCH = HW // CCH   # 18
MASK_VAL = -1.0e5


def _split_waits(nc, limit=1):
    """Walrus in this environment accepts exactly one sync-wait command per
    instruction; Tile emits several. Move excess waits onto NoOps inserted
    just before, on the same engine."""
    n_added = 0
    for fn in nc.m.functions:
        for bb in fn.blocks:
            insts = bb.instructions
            idx = 0
            while idx < len(insts):
                inst = insts[idx]
                si = inst.sync_info
                waits = list(si.on_wait) if si and si.on_wait else []
                if len(waits) > limit:
                    keep = waits[-limit:]
                    extra = waits[:-limit]
                    pos = idx
                    for j in range(0, len(extra), limit):
                        chunk = extra[j : j + limit]
                        nop = mybir.InstNoOp(name=f"I-wsplit-{n_added}")
                        n_added += 1
                        nop.engine = inst.engine
                        nop.sync_info = mybir.SyncInfo(on_wait=chunk, on_update=[])
                        insts.insert(pos, nop)
                        pos += 1
                        idx += 1
                    inst.sync_info = mybir.SyncInfo(
                        on_wait=keep, on_update=list(si.on_update or [])
                    )
                idx += 1
    return n_added


def build_program(gamma: float, split_waits: bool = True, reps: int = 1) -> bass.Bass:
    nc = bass.Bass()

    qb = nc.declare_dram_parameter("qb", [C, HW], BF16, isOutput=False)
    vb = nc.declare_dram_parameter("vb", [C, HW], BF16, isOutput=False)
    wq = nc.declare_dram_parameter("wq", [C, 256], BF16, isOutput=False)
    wk = nc.declare_dram_parameter("wk", [C, 256], BF16, isOutput=False)
    wv = nc.declare_dram_parameter("wv", [C, 256], BF16, isOutput=False)
    bq = nc.declare_dram_parameter("bq", [256], F32, isOutput=False)
    bk = nc.declare_dram_parameter("bk", [256], F32, isOutput=False)
    bv = nc.declare_dram_parameter("bv", [256], F32, isOutput=False)
    ones96 = nc.declare_dram_parameter("ones96", [96, 1], BF16, isOutput=False)
    ones1x64 = nc.declare_dram_parameter("ones1x64", [1, 64], BF16, isOutput=False)
    eye128 = nc.declare_dram_parameter("eye128", [128, 128], BF16, isOutput=False)
    eye96f = nc.declare_dram_parameter("eye96f", [96, 96], F32, isOutput=False)
    negeye96 = nc.declare_dram_parameter("negeye96", [96, 96], BF16, isOutput=False)
    ipat1 = nc.declare_dram_parameter("ipat1", [96, 384], BF16, isOutput=False)
    outp = nc.declare_dram_parameter("out", [256, HW], F32, isOutput=True)

    qb_r = qb[:].rearrange("(k p) n -> p k n", p=128)
    vb_r = vb[:].rearrange("(k p) n -> p k n", p=128)
    out_r = outp[:].rearrange("(m p) n -> p m n", p=128)

    with TileContext(nc) as tc:
        with (
            tc.tile_pool(name="cpool", bufs=1) as cpool,
            tc.tile_pool(name="big", bufs=1) as big,
            tc.tile_pool(name="small", bufs=1) as small,
            tc.tile_pool(name="spool", bufs=3) as spool,
            tc.tile_pool(name="apool", bufs=2) as apool,
            tc.tile_pool(name="epool", bufs=4) as epool,
        ):
            wq_sb = cpool.tile_from(wq[:].rearrange("(k p) m -> p k m", p=128))
            wk_sb = cpool.tile_from(wk[:].rearrange("(k p) m -> p k m", p=128))
            wv_sb = cpool.tile_from(wv[:].rearrange("(k p) m -> p k m", p=128))
            bq_sb0 = cpool.tile_from(bq[:].rearrange("(m p) -> p m", p=128))
            bk_sb0 = cpool.tile_from(bk[:].rearrange("(m p) -> p m", p=128))
            bv_sb0 = cpool.tile_from(bv[:].rearrange("(m p) -> p m", p=128))
            bq_sb = cpool.tile([128, 2], F32, name="bq_c")
            bk_sb = cpool.tile([128, 2], F32, name="bk_c")
            bv_sb = cpool.tile([128, 2], F32, name="bv_c")
            nc.vector.tensor_copy(bq_sb[:], bq_sb0[:])
            nc.vector.tensor_copy(bk_sb[:], bk_sb0[:])
            nc.vector.tensor_copy(bv_sb[:], bv_sb0[:])
            o96_sb = cpool.tile_from(ones96[:])
            o1x64_sb = cpool.tile_from(ones1x64[:])
            eye128_sb = cpool.tile_from(eye128[:])
            eye96_sb = cpool.tile_from(eye96f[:])
            neye_sb = cpool.tile_from(negeye96[:])
            ipat_sb = cpool.tile_from(ipat1[:])

            for rep_mt in range(2 * reps):
                mt = rep_mt % 2
                # ---------------- conv phase ----------------
                t_sb = big.tile([128, HW], BF16, tag="t", name=f"t{mt}")
                f_sb = big.tile([128, HW], BF16, tag="f", name=f"f{mt}")
                g_sb = big.tile([128, HW], BF16, tag="g", name=f"g{mt}")
                with tc.tile_pool(name=f"cvps{mt}", bufs=4, space="PSUM") as cvps:
                    for ch in range(NCH):
                        sl = slice(ch * CCH, (ch + 1) * CCH)
                        qc = spool.tile([128, 4, CCH], BF16, tag="qc", name=f"qc{mt}_{ch}")
                        nc.sync.dma_start(out=qc[:], in_=qb_r[:, :, sl])
                        vc = spool.tile([128, 4, CCH], BF16, tag="vc", name=f"vc{mt}_{ch}")
                        nc.gpsimd.dma_start(out=vc[:], in_=vb_r[:, :, sl])
                        for w_sb, b_sb, src, dst in (
                            (wq_sb, bq_sb, qc, t_sb),
                            (wk_sb, bk_sb, qc, f_sb),
                            (wv_sb, bv_sb, vc, g_sb),
                        ):
                            ps = cvps.tile([128, CCH], F32, tag="cv", name=f"cv{mt}_{ch}")
                            for k in range(4):
                                nc.tensor.matmul(
                                    ps[:],
                                    w_sb[:, k, mt * 128 : (mt + 1) * 128],
                                    src[:, k, :],
                                    start=(k == 0),
                                    stop=(k == 3),
                                )
                            nc.scalar.activation(
                                dst[:, sl], ps[:], AF.Identity,
                                bias=b_sb[:, mt : mt + 1],
                            )

                # ---------------- gT phase ----------------
                # transpose both heads at once: in [128d, 96] -> out [96, 128]
                gtr = [
                    big.tile([96, H * D], BF16, tag=f"gtr{h}", name=f"gtr{mt}_{h}")
                    for h in range(2)
                ]
                gtc = [
                    big.tile([96, H * D], BF16, tag=f"gtc{h}", name=f"gtc{mt}_{h}")
                    for h in range(2)
                ]
                with tc.tile_pool(name=f"gtps{mt}", bufs=4, space="PSUM") as gtps:
                    for orient in range(2):  # 0=row (per y), 1=col (per x)
                        dsts = gtr if orient == 0 else gtc
                        for blk in range(H // 4):  # 4 transposes per bank
                            ps = gtps.tile([96, 512], BF16, tag="gt", name=f"gt{mt}_{orient}_{blk}")
                            for tix in range(4):
                                yx = blk * 4 + tix
                                if orient == 0:
                                    src = g_sb[:, yx * 96 : (yx + 1) * 96]
                                else:
                                    src = g_sb[:, yx : HW : 96]
                                nc.tensor.transpose(
                                    ps[:, tix * 128 : (tix + 1) * 128], src, eye128_sb[:]
                                )
                            for h in range(2):
                                pv = ps[:].rearrange("p (t q) -> p t q", t=4)[
                                    :, :, h * 64 : (h + 1) * 64
                                ]
                                dv = dsts[h][:, blk * 256 : (blk + 1) * 256].rearrange(
                                    "p (t q) -> p t q", t=4
                                )
                                nc.vector.tensor_copy(dv, pv)

                # ---------------- attention ----------------
                orc = big.tile([128, HW], BF16, tag="orc", name=f"orc{mt}")
                with tc.tile_pool(name=f"aps{mt}", bufs=1, space="PSUM") as aps:
                    # --- row branch ---
                    ztr = [
                        aps.tile([96, 96], F32, tag="z", bufs=2, name=f"ztr{mt}_{h}")
                        for h in range(2)
                    ]
                    for grp in range(NG):
                        eps = aps.tile([96, 1024], F32, tag="e", bufs=2, name=f"er_ps{mt}_{grp}")
                        for h in range(2):
                            for j in range(G):
                                y = grp * G + j
                                sl = slice(y * 96, (y + 1) * 96)
                                nc.tensor.matmul(
                                    eps[:, h * 512 + j * 96 : h * 512 + (j + 1) * 96],
                                    f_sb[h * 64 : (h + 1) * 64, sl],
                                    t_sb[h * 64 : (h + 1) * 64, sl],
                                    start=True, stop=True,
                                )
                        er = apool.tile([96, 768], BF16, tag="er", name=f"er{mt}_{grp}")
                        nc.scalar.activation(
                            er[:].rearrange("p (b n) -> p b n", b=2),
                            eps[:].rearrange("p (b n) -> p b n", b=2)[:, :, 0:384],
                            AF.Exp,
                        )
                        ops_ = aps.tile([128, 384], F32, tag="o", bufs=2, name=f"or_ps{mt}_{grp}")
                        for h in range(2):
                            for j in range(G):
                                y = grp * G + j
                                esl = slice(h * 384 + j * 96, h * 384 + (j + 1) * 96)
                                nc.tensor.matmul(
                                    ops_[h * 64 : (h + 1) * 64, j * 96 : (j + 1) * 96],
                                    gtr[h][:, y * D : (y + 1) * D],
                                    er[:, esl],
                                    start=True, stop=True,
                                )
                                nc.tensor.matmul(
                                    ztr[h][:, y : y + 1],
                                    er[:, esl],
                                    o96_sb[:],
                                    start=True, stop=True,
                                    skip_group_check=True,
                                )
                        nc.vector.tensor_copy(orc[:, grp * 384 : (grp + 1) * 384], ops_[:])

                    zr_sb = [
                        small.tile([96, 96], F32, tag=f"zr{h}", name=f"zr{mt}_{h}")
                        for h in range(2)
                    ]
                    for h in range(2):
                        nc.vector.tensor_copy(zr_sb[h][:], ztr[h][:])

                    # --- col branch ---
                    zm = [
                        aps.tile([96, 96], F32, tag="z", bufs=2, name=f"zm{mt}_{h}")
                        for h in range(2)
                    ]
                    for h in range(2):
                        nc.tensor.transpose(zm[h][:], zr_sb[h][:], eye96_sb[:])
                    orc_x = orc[:].rearrange("p (y x) -> p x y", x=96)
                    for grp in range(NG):
                        eps = aps.tile([96, 1024], F32, tag="e", bufs=2, name=f"ec_ps{mt}_{grp}")
                        for h in range(2):
                            nc.tensor.matmul(
                                eps[:, h * 512 : h * 512 + 384],
                                neye_sb[:], ipat_sb[:],
                                start=True, stop=False,
                                skip_group_check=True,
                            )
                            for j in range(G):
                                x = grp * G + j
                                nc.tensor.matmul(
                                    eps[:, h * 512 + j * 96 : h * 512 + (j + 1) * 96],
                                    f_sb[h * 64 : (h + 1) * 64, x : HW : 96],
                                    t_sb[h * 64 : (h + 1) * 64, x : HW : 96],
                                    start=False, stop=True,
                                    skip_group_check=True,
                                )
                        ec = apool.tile([96, 768], BF16, tag="er", name=f"ec{mt}_{grp}")
                        nc.scalar.activation(
                            ec[:].rearrange("p (b n) -> p b n", b=2),
                            eps[:].rearrange("p (b n) -> p b n", b=2)[:, :, 0:384],
                            AF.Exp,
                        )
                        ops_ = aps.tile([128, 384], F32, tag="o", bufs=2, name=f"oc_ps{mt}_{grp}")
                        for h in range(2):
                            for j in range(G):
                                x = grp * G + j
                                esl = slice(h * 384 + j * 96, h * 384 + (j + 1) * 96)
                                nc.tensor.matmul(
                                    ops_[h * 64 : (h + 1) * 64, j * 96 : (j + 1) * 96],
                                    gtc[h][:, x * D : (x + 1) * D],
                                    ec[:, esl],
                                    start=True, stop=True,
                                )
                                nc.tensor.matmul(
                                    zm[h][:, x : x + 1],
                                    ec[:, esl],
                                    o96_sb[:],
                                    start=False, stop=True,
                                    skip_group_check=True,
                                )
                        dv = orc_x[:, grp * G : (grp + 1) * G, :]
                        nc.vector.tensor_tensor(
                            out=dv,
                            in0=ops_[:].rearrange("p (j n) -> p j n", j=G),
                            in1=dv,
                            op=ALU.add,
                        )

                    # --- Z finalize ---
                    zflat = [
                        small.tile([1, HW], BF16, tag=f"zf{h}", name=f"zf{mt}_{h}")
                        for h in range(2)
                    ]
                    for h in range(2):
                        zi_f = small.tile([96, 96], F32, tag="zi_f", name=f"zi_f{mt}_{h}")
                        nc.vector.reciprocal(zi_f[:], zm[h][:])
                        zi_b = small.tile([96, 96], BF16, tag="zi_b", name=f"zi_b{mt}_{h}")
                        nc.vector.tensor_scalar_mul(zi_b[:], zi_f[:], float(gamma))
                        nc.sync.dma_start(out=zflat[h][:], in_=zi_b[:])

                # ---------------- epilogue ----------------
                with tc.tile_pool(name=f"bps{mt}", bufs=2, space="PSUM") as bps:
                    for grp in range(NG):
                        sl = slice(grp * 384, (grp + 1) * 384)
                        pb = bps.tile([128, 384], F32, tag="b", name=f"pb{mt}_{grp}")
                        for h in range(2):
                            nc.tensor.matmul(
                                pb[h * 64 : (h + 1) * 64, :],
                                o1x64_sb[:],
                                zflat[h][0:1, sl],
                                start=True, stop=True,
                            )
                        on = epool.tile([128, 384], F32, tag="on", name=f"on{mt}_{grp}")
                        nc.vector.tensor_tensor(
                            out=on[:], in0=orc[:, sl], in1=pb[:], op=ALU.mult
                        )
                        nc.gpsimd.dma_start(out=out_r[:, mt, sl], in_=on[:])

    if split_waits:
        _split_waits(nc)
    return nc


def make_in_maps(q, v, Wq, bq, Wk, bk, Wv, bv):
    """Build the 8 per-core input dicts (host-side sharding + dtype prep)."""
    consts = {
        "ones96": np.ones((96, 1), BF),
        "ones1x64": np.ones((1, 64), BF),
        "eye128": np.eye(128, dtype=BF),
        "eye96f": np.eye(96, dtype=np.float32),
        "negeye96": (MASK_VAL * np.eye(96)).astype(BF),
        "ipat1": np.hstack([np.eye(96, dtype=BF)] * 4),
    }
    in_maps = []
    for core in range(8):
        b, hg = core // 2, core % 2
        ch0 = hg * 256
        m = dict(consts)
        m["qb"] = np.ascontiguousarray(q[b].reshape(C, HW)).astype(BF)
        m["vb"] = np.ascontiguousarray(v[b].reshape(C, HW)).astype(BF)
        m["wq"] = np.ascontiguousarray((0.125 * Wq[ch0 : ch0 + 256]).T).astype(BF)
        m["wk"] = np.ascontiguousarray(Wk[ch0 : ch0 + 256].T).astype(BF)
        m["wv"] = np.ascontiguousarray(Wv[ch0 : ch0 + 256].T).astype(BF)
        m["bq"] = np.ascontiguousarray(0.125 * bq[ch0 : ch0 + 256]).astype(np.float32)
        m["bk"] = np.ascontiguousarray(bk[ch0 : ch0 + 256]).astype(np.float32)
        m["bv"] = np.ascontiguousarray(bv[ch0 : ch0 + 256]).astype(np.float32)
        in_maps.append(m)
    return in_maps


def assemble(results, v):
    out = np.empty((B, C, H, W), np.float32)
    for core in range(8):
        b, hg = core // 2, core % 2
        ch0 = hg * 256
        out[b, ch0 : ch0 + 256] = np.asarray(results[core]["out"], np.float32).reshape(
            256, H, W
        ) + v[b, ch0 : ch0 + 256]
    return out


def kernel(q, v, Wq, bq, Wk, bk, Wv, bv, gamma, _trace=False):
    from concourse.bass_utils import run_bass_kernel_spmd

    q = np.asarray(q, np.float32)
    v = np.asarray(v, np.float32)
    nc = build_program(float(np.asarray(gamma).reshape(-1)[0]))
    in_maps = make_in_maps(
        q, v,
        np.asarray(Wq, np.float32), np.asarray(bq, np.float32),
        np.asarray(Wk, np.float32), np.asarray(bk, np.float32),
        np.asarray(Wv, np.float32), np.asarray(bv, np.float32),
    )
    res = run_bass_kernel_spmd(nc, in_maps, list(range(8)))
    out = assemble(res.results, v)
    if _trace:
        return out, res
    return out
